# revision 4
# baseline (speedup 1.0000x reference)
"""Trainium2 Bass kernel for a 2-layer LSTM decoder (nn_Decoder_recurrent).

Strategy (8 NeuronCores, data-parallel over batch):
  - Each core handles B_local = 1024/8 = 128 batch rows for all T=336 steps.
  - All weights SBUF-resident. The two recurrent matmuls (h0 @ W_hh0.T and
    h1 @ W_hh1.T) run in fp8e4m3 DoubleRow mode (K=256 per instruction,
    2x PE throughput); W_ih1 stays fp16 to keep the rel-err within budget
    (measured in simulation: fp8 on whh0+whh1 -> 1.34e-2, all-fp8 -> 2.0e-2).
  - fp8 weights are scaled x256 to stay in e4m3 normal range; the ACT
    gate activations descale with scale=1/256. The fp16 x/bias path (k9)
    and W_ih1 are also stored x256 so all gate PSUMs share one scale.
  - h states: kept transposed in both fp16 ([128,KT,BL], for fp16 matmuls /
    y-proj) and fp8 (same layout, for DoubleRow stationaries). PE transposes
    fp16; DVE copies PSUM->SBUF fp16; ACT converts fp16->fp8.
  - Software pipelining: the DoubleRow part of step t+1's layer-0 gates for
    groups 0..3 is emitted before step t's h1-transpose/y-projection, so the
    PE has independent work while the layer-1 cell math completes.
"""

import sys
import time

sys.path.insert(0, '/opt/trn_rl_repo')

import numpy as np
import ml_dtypes

import concourse.bass as bass
import concourse.tile as tile
from concourse import mybir
import bass_rust
from concourse.bass_utils import run_bass_kernel_spmd
from concourse.masks import make_identity

B, T, M, Q = 1024, 336, 1, 9
DFF, H, L = 7, 1024, 2
NCORES = 8
BL = B // NCORES        # 128 batch rows per core
KT = H // 128           # 8 K-tiles per hidden matmul
KP = KT // 2            # 4 K-pairs for fp8 DoubleRow (K=256 each)
G4H = 4 * H             # 4096 gate columns
NG = 512                # gate psum group width (one PSUM bank of fp32)
NGROUPS = G4H // NG     # 8
WSCALE = 256.0          # fp8 weight scale (keeps e4m3 out of subnormals)

E4 = ml_dtypes.float8_e4m3
f8 = mybir.dt.float8e4
f16 = mybir.dt.float16
f32 = mybir.dt.float32
DR = mybir.MatmulPerfMode.DoubleRow

SIG = mybir.ActivationFunctionType.Sigmoid
TANH = mybir.ActivationFunctionType.Tanh
IDENT = mybir.ActivationFunctionType.Identity
COPY = mybir.ActivationFunctionType.Copy

_module_cache = {}
_exec_cache = {}


def _split_multi_waits(nc, max_waits=1):
    """This container's walrus accepts at most one sem-wait per instruction;
    hoist extras onto same-engine NoOps placed immediately before."""
    for f in nc.m.functions:
        for bb in f.blocks:
            new_insts = []
            for inst in bb.instructions:
                si = inst.sync_info
                if si is not None and si.on_wait and len(si.on_wait) > max_waits:
                    waits = list(si.on_wait)
                    for j, w in enumerate(waits[max_waits:]):
                        nop = bass_rust.InstNoOp(
                            name=f"{inst.name}-sw{j}", ins=[], outs=[])
                        nop.engine = inst.engine
                        nop.sync_info = mybir.SyncInfo(on_wait=[w], on_update=[])
                        new_insts.append(nop)
                    si.on_wait = waits[:max_waits]
                new_insts.append(inst)
            bb.instructions = new_insts


def _build_module(Tsteps):
    nc = bass.Bass("TRN2", target_bir_lowering=False)

    d_whh0 = nc.dram_tensor("whh0q", [128, KT, G4H], f8, kind="ExternalInput")
    d_whh1 = nc.dram_tensor("whh1q", [128, KT, G4H], f8, kind="ExternalInput")
    d_wih1 = nc.dram_tensor("wih1t", [128, KT, G4H], f16, kind="ExternalInput")
    # k9pack rows: 0 = W_ih0[:,0] (y weight), 1-7 = W_ih0[:,1:8].T,
    # 8 = b_ih0+b_hh0, 9 = b_ih1+b_hh1, 10-127 = zeros.  All x256.
    d_k9 = nc.dram_tensor("k9pack", [128, G4H], f16, kind="ExternalInput")
    d_wout = nc.dram_tensor("woutt", [128, KT, Q], f16, kind="ExternalInput")
    d_bout = nc.dram_tensor("bout", [Q, 1], f32, kind="ExternalInput")
    d_xbias = nc.dram_tensor("xbias", [128, 128], f16, kind="ExternalInput")
    d_h0t16 = nc.dram_tensor("h0t16", [128, KT, BL], f16, kind="ExternalInput")
    d_h1t16 = nc.dram_tensor("h1t16", [128, KT, BL], f16, kind="ExternalInput")
    d_h0t8 = nc.dram_tensor("h0t8", [128, KT, BL], f8, kind="ExternalInput")
    d_h1t8 = nc.dram_tensor("h1t8", [128, KT, BL], f8, kind="ExternalInput")
    d_c = nc.dram_tensor("cinit", [L, BL, H], f32, kind="ExternalInput")
    # ffy rows: 0 = y_prev slot (host fills t=0 only), 1-7 = f_t, 8 = ones
    d_ffy = nc.dram_tensor("ffy", [Tsteps, 9, BL], f16, kind="ExternalInput")
    d_y = nc.dram_tensor("yout", [Tsteps, Q, BL], f32, kind="ExternalOutput")

    with tile.TileContext(nc) as tc:
        with (
            tc.tile_pool(name="wres", bufs=1) as wres,
            tc.tile_pool(name="state", bufs=1) as state,
            tc.tile_pool(name="acttmp", bufs=1) as acttmp,
            tc.tile_pool(name="dvetmp", bufs=1) as dvetmp,
            tc.tile_pool(name="ytp", bufs=3) as ytp,
            tc.tile_pool(name="gpsum", bufs=5, space="PSUM") as gpsum,
            tc.tile_pool(name="tpsum", bufs=2, space="PSUM") as tpsum,
            tc.tile_pool(name="ypsum", bufs=1, space="PSUM") as ypsum,
        ):
            w_hh0 = wres.tile([128, KT, G4H], f8, tag="w_hh0")
            w_hh1 = wres.tile([128, KT, G4H], f8, tag="w_hh1")
            w_ih1 = wres.tile([128, KT, G4H], f16, tag="w_ih1")
            k9 = wres.tile([128, G4H], f16, tag="k9")
            w_out = wres.tile([128, KT, Q], f16, tag="w_out")
            b_out = wres.tile([Q, 1], f32, tag="b_out")
            ident = wres.tile([128, 128], f16, tag="ident")
            nc.sync.dma_start(w_hh0[:], d_whh0[:])
            nc.sync.dma_start(w_hh1[:], d_whh1[:])
            nc.sync.dma_start(w_ih1[:], d_wih1[:])
            nc.sync.dma_start(k9[:], d_k9[:])
            nc.sync.dma_start(w_out[:], d_wout[:])
            nc.sync.dma_start(b_out[:], d_bout[:])
            make_identity(nc, ident[:])

            h0T16 = state.tile([128, KT, BL], f16, tag="h0T16")
            h1T16 = state.tile([128, KT, BL], f16, tag="h1T16")
            h0T8 = state.tile([128, KT, BL], f8, tag="h0T8")
            h1T8 = state.tile([128, KT, BL], f8, tag="h1T8")
            c0 = state.tile([BL, H], f32, tag="c0")
            c1 = state.tile([BL, H], f32, tag="c1")
            nc.sync.dma_start(h0T16[:], d_h0t16[:])
            nc.sync.dma_start(h1T16[:], d_h1t16[:])
            nc.sync.dma_start(h0T8[:], d_h0t8[:])
            nc.sync.dma_start(h1T8[:], d_h1t8[:])
            nc.sync.dma_start(c0[:], d_c[0])
            nc.sync.dma_start(c1[:], d_c[1])

            # bias-only stationary for layer 1: row 9 = ones, rest zeros
            xbias = state.tile([128, 128], f16, tag="xbias")
            nc.sync.dma_start(xbias[:], d_xbias[:])

            # rotating per-step input stationaries (rows 9-127 stay zero)
            xaug = [state.tile([128, BL], f16, tag=f"xaug{i}", name=f"xaug{i}")
                    for i in range(3)]
            for xt in xaug:
                nc.vector.memset(xt[:], 0.0)

            # fp16 activation temps (full gate blocks) and fp32 cell temp
            si = [acttmp.tile([BL, H], f16, tag=f"si{l}", name=f"si{l}")
                  for l in range(2)]
            sf = [acttmp.tile([BL, H], f16, tag=f"sf{l}", name=f"sf{l}")
                  for l in range(2)]
            tg = [acttmp.tile([BL, H], f16, tag=f"tg{l}", name=f"tg{l}")
                  for l in range(2)]
            so = [acttmp.tile([BL, H], f16, tag=f"so{l}", name=f"so{l}")
                  for l in range(2)]
            tc_ = [acttmp.tile([BL, H], f16, tag=f"tc{l}", name=f"tc{l}")
                   for l in range(2)]
            hn = [acttmp.tile([BL, H], f16, tag=f"hn{l}", name=f"hn{l}")
                  for l in range(2)]
            t1 = dvetmp.tile([BL, H], f32, tag="t1")

            cs = [c0, c1]

            def act_for_group(layer, g, ps):
                """Descale + nonlinearity for gate psum group g -> fp16."""
                blk, half = divmod(g, 2)
                dst = (si, sf, tg, so)[blk][layer]
                func = TANH if blk == 2 else SIG
                nc.scalar.activation(
                    dst[:, half * NG:(half + 1) * NG], ps[:], func,
                    scale=1.0 / WSCALE)

            def cell_math(layer):
                """c = sig(f)*c + sig(i)*tanh(g); h = sig(o)*tanh(c)."""
                c = cs[layer]
                nc.vector.tensor_tensor(t1[:], si[layer][:], tg[layer][:],
                                        mybir.AluOpType.mult)
                nc.vector.tensor_tensor(c[:], c[:], sf[layer][:],
                                        mybir.AluOpType.mult)
                nc.vector.tensor_tensor(c[:], c[:], t1[:],
                                        mybir.AluOpType.add)
                nc.scalar.activation(tc_[layer][:, 0:NG], c[:, 0:NG], TANH)
                nc.scalar.activation(tc_[layer][:, NG:2 * NG], c[:, NG:2 * NG],
                                     TANH)
                nc.vector.tensor_tensor(hn[layer][:], so[layer][:],
                                        tc_[layer][:], mybir.AluOpType.mult)

            def transpose_h(layer, hT16, hT8):
                """PE transpose hn -> PSUM fp16; DVE copy -> SBUF fp16;
                ACT convert -> SBUF fp8 (if hT8 is not None)."""
                tp = tpsum.tile([128, KT, BL], f16, tag="tp")
                for j in range(KT):
                    nc.tensor.transpose(tp[:, j, :],
                                        hn[layer][:, j * 128:(j + 1) * 128],
                                        ident[:])
                nc.vector.tensor_copy(hT16[:], tp[:])
                if hT8 is not None:
                    nc.scalar.activation(hT8[:], hT16[:], COPY)

            def emit_l0_dr(g):
                """DoubleRow part of layer-0 gates for group g (no stop)."""
                ps = gpsum.tile([BL, NG], f32, tag="gps")
                for kp in range(KP):
                    nc.tensor.matmul(ps[:], h0T8[:, 2 * kp:2 * kp + 2, :],
                                     w_hh0[:, 2 * kp:2 * kp + 2,
                                           g * NG:(g + 1) * NG],
                                     start=(kp == 0), stop=False,
                                     perf_mode=DR)
                return ps

            def emit_l0_k9(g, ps, xa):
                """x/bias contribution closing layer-0 group g."""
                nc.tensor.matmul(ps[:], xa[:], k9[:, g * NG:(g + 1) * NG],
                                 start=False, stop=True)
                act_for_group(0, g, ps)

            g1ps = [None] * NGROUPS

            def emit_A(g):
                ps = gpsum.tile([BL, NG], f32, tag="gps")
                g1ps[g] = ps
                for kp in range(KP):
                    nc.tensor.matmul(ps[:], h1T8[:, 2 * kp:2 * kp + 2, :],
                                     w_hh1[:, 2 * kp:2 * kp + 2,
                                           g * NG:(g + 1) * NG],
                                     start=(kp == 0), stop=False,
                                     perf_mode=DR)

            def emit_B(g):
                ps = g1ps[g]
                for k in range(KT):
                    nc.tensor.matmul(ps[:], h0T16[:, k, :],
                                     w_ih1[:, k, g * NG:(g + 1) * NG],
                                     start=False, stop=False)
                nc.tensor.matmul(ps[:], xbias[:],
                                 k9[:, g * NG:(g + 1) * NG],
                                 start=False, stop=True)
                act_for_group(1, g, ps)

            yts_prev = None
            l0_pending = [None] * NGROUPS   # psum handles from prior iter
            for t in range(Tsteps):
                xa = xaug[t % 3]
                if t == 0:
                    nc.sync.dma_start(xa[0:9, :], d_ffy[t, 0:9, :])
                else:
                    nc.sync.dma_start(xa[1:9, :], d_ffy[t, 1:9, :])
                    nc.vector.tensor_copy(xa[0:1, :], yts_prev[0:1, :])

                # ---- layer 0 gates: close pipelined groups, emit the rest
                for g in range(NGROUPS):
                    if l0_pending[g] is None:
                        l0_pending[g] = emit_l0_dr(g)
                    emit_l0_k9(g, l0_pending[g], xa)
                    l0_pending[g] = None

                cell_math(0)

                # ---- layer 1 gates: A = h1 @ W_hh1.T (fp8), B = h0new @
                # W_ih1.T + b1 (fp16). A(0..3) covers PE while cell0 runs.
                for g in range(4):
                    emit_A(g)
                transpose_h(0, h0T16, h0T8)
                for g in range(4):
                    emit_A(g + 4)
                    emit_B(g)
                for g in range(4, NGROUPS):
                    emit_B(g)

                cell_math(1)

                # ---- pipeline: open next step's layer-0 DR groups 0..3 so
                # the PE has work while cell1's ACT/DVE chain completes.
                if t + 1 < Tsteps:
                    for g in range(4):
                        l0_pending[g] = emit_l0_dr(g)

                transpose_h(1, h1T16, h1T8)

                # ---- output projection: y^T = W_out @ h1^T + b_out
                yp = ypsum.tile([Q, BL], f32, tag="yp")
                for k in range(KT):
                    nc.tensor.matmul(yp[:], w_out[:, k, :], h1T16[:, k, :],
                                     start=(k == 0), stop=(k == KT - 1))
                yts = ytp.tile([Q, BL], f32, tag="yts")
                nc.scalar.activation(yts[:], yp[:], IDENT, bias=b_out[:, 0:1])
                nc.sync.dma_start(d_y[t], yts[:])
                yts_prev = yts

    _split_multi_waits(nc)
    return nc


def _pack_weights(inputs):
    def t_pack16(w, scale=1.0):  # W [4H, K] -> [128, K/128, 4H] fp16
        wt = np.ascontiguousarray(w.T.astype(np.float32) * scale)
        k = wt.shape[0]
        return np.ascontiguousarray(
            wt.reshape(k // 128, 128, wt.shape[1]).transpose(1, 0, 2)
        ).astype(np.float16)

    def t_pack8(w):  # W [4H, K] -> [128, K/128, 4H] fp8 (x256)
        wt = np.ascontiguousarray(w.T.astype(np.float32) * WSCALE)
        k = wt.shape[0]
        return np.ascontiguousarray(
            wt.reshape(k // 128, 128, wt.shape[1]).transpose(1, 0, 2)
        ).astype(E4)

    whh0q = t_pack8(np.asarray(inputs["W_hh0"], np.float32))
    whh1q = t_pack8(np.asarray(inputs["W_hh1"], np.float32))
    wih1t = t_pack16(np.asarray(inputs["W_ih1"], np.float32), WSCALE)

    k9 = np.zeros((128, G4H), np.float32)
    W_ih0 = np.asarray(inputs["W_ih0"], np.float32)  # [4H, 8]
    k9[0, :] = W_ih0[:, 0]
    k9[1:8, :] = W_ih0[:, 1:8].T
    k9[8, :] = np.asarray(inputs["b_ih0"], np.float32) + np.asarray(
        inputs["b_hh0"], np.float32)
    k9[9, :] = np.asarray(inputs["b_ih1"], np.float32) + np.asarray(
        inputs["b_hh1"], np.float32)
    k9 = (k9 * WSCALE).astype(np.float16)

    woutT = np.asarray(inputs["W_out"], np.float32).T  # [H, 9]
    woutt = np.ascontiguousarray(
        woutT.reshape(KT, 128, Q).transpose(1, 0, 2)).astype(np.float16)
    bout = np.asarray(inputs["b_out"], np.float32).reshape(Q, 1)
    return whh0q, whh1q, wih1t, k9, woutt, bout


def _make_in_maps(inputs, Tsteps):
    whh0q, whh1q, wih1t, k9, woutt, bout = _pack_weights(inputs)

    h = np.asarray(inputs["h"], np.float32)     # [2, B, H]
    c = np.asarray(inputs["c"], np.float32)
    ff = np.asarray(inputs["future_features"], np.float32)[:, :Tsteps]
    y0 = np.asarray(inputs["inp_y"], np.float32)[:, 0, 0]   # [B]

    xbias_np = np.zeros((128, 128), np.float16)
    xbias_np[9, :] = 1.0

    in_maps = []
    for core in range(NCORES):
        s = slice(core * BL, (core + 1) * BL)

        def h_pack(hl):  # h [BL, H] -> h.T -> [128, KT, BL] fp16
            ht = np.ascontiguousarray(hl.T)
            return np.ascontiguousarray(
                ht.reshape(KT, 128, BL).transpose(1, 0, 2)).astype(np.float16)

        h0t16 = h_pack(h[0, s])
        h1t16 = h_pack(h[1, s])

        ffy = np.zeros((Tsteps, 9, BL), np.float32)
        ffy[0, 0, :] = y0[s]
        ffy[:, 1:8, :] = ff[s].transpose(1, 2, 0)  # [T, 7, BL]
        ffy[:, 8, :] = 1.0

        in_maps.append({
            "whh0q": whh0q,
            "whh1q": whh1q,
            "wih1t": wih1t,
            "k9pack": k9,
            "woutt": woutt,
            "bout": bout,
            "xbias": xbias_np,
            "h0t16": h0t16,
            "h1t16": h1t16,
            "h0t8": h0t16.astype(E4),
            "h1t8": h1t16.astype(E4),
            "cinit": np.ascontiguousarray(c[:, s, :]),
            "ffy": ffy.astype(np.float16),
        })
    return in_maps


def _unpack_out(res_list, Tsteps):
    out = np.empty((B, Tsteps, Q), np.float32)
    for core in range(NCORES):
        s = slice(core * BL, (core + 1) * BL)
        out[s] = res_list[core]["yout"].transpose(2, 0, 1)  # [BL, T, 9]
    return out.reshape(B, Tsteps, M, Q)


def kernel(**inputs):
    return _run(inputs, T)


def _run(inputs, Tsteps, trace=False):
    if Tsteps not in _module_cache:
        _module_cache[Tsteps] = _build_module(Tsteps)
    nc = _module_cache[Tsteps]
    in_maps = _make_in_maps(inputs, Tsteps)
    res = run_bass_kernel_spmd(nc, in_maps, core_ids=list(range(NCORES)),
                               trace=trace)
    _run.last_result = res
    return _unpack_out(res.results, Tsteps)


# ---------------------------------------------------------------- benching

def _build_exec(nc):
    """Persistent jitted executor (run_bass_via_pjrt builds a fresh jit per
    call; for benching we need one we can invoke repeatedly)."""
    import jax
    from jax.sharding import Mesh, PartitionSpec, NamedSharding
    from jax.experimental.shard_map import shard_map
    from concourse import bass2jax
    from concourse.bass2jax import (_bass_exec_p, install_neuronx_cc_hook,
                                    partition_id_tensor)

    install_neuronx_cc_hook()

    partition_name = (nc.partition_id_tensor.name
                      if nc.partition_id_tensor else None)
    in_names, out_names, out_avals, zero_outs = [], [], [], []
    for alloc in nc.m.functions[0].allocations:
        if not isinstance(alloc, mybir.MemoryLocationSet):
            continue
        name = alloc.memorylocations[0].name
        if alloc.kind == "ExternalInput":
            if name == partition_name:
                continue
            in_names.append(name)
        elif alloc.kind == "ExternalOutput":
            out_names.append(name)
            shape = tuple(alloc.tensor_shape)
            dtype = mybir.dt.np(alloc.dtype)
            out_avals.append(jax.core.ShapedArray(shape, dtype))
            zero_outs.append(np.zeros(shape, dtype))
    n_params = len(in_names)
    all_names = in_names + out_names
    if partition_name is not None:
        all_names = all_names + [partition_name]

    def _body(*args):
        operands = list(args)
        if partition_name is not None:
            operands.append(partition_id_tensor())
        outs = _bass_exec_p.bind(
            *operands,
            out_avals=tuple(out_avals),
            in_names=tuple(all_names),
            out_names=tuple(out_names),
            lowering_input_output_aliases=(),
            sim_require_finite=True,
            sim_require_nnan=True,
            nc=nc,
        )
        return tuple(outs)

    devices = jax.devices()[:NCORES]
    mesh = Mesh(np.asarray(devices), ("core",))
    spec = PartitionSpec("core")
    n_outs = len(out_names)
    donate = tuple(range(n_params, n_params + n_outs))
    fn = jax.jit(
        shard_map(_body, mesh=mesh,
                  in_specs=(spec,) * (n_params + n_outs),
                  out_specs=(spec,) * n_outs, check_rep=False),
        donate_argnums=donate, keep_unused=True)
    sharding = NamedSharding(mesh, spec)
    return fn, in_names, out_names, zero_outs, sharding


def bench(inputs, Tsteps=T, iters=5):
    """Returns (out, per-call wall ns list). First call compiles."""
    import jax
    if Tsteps not in _module_cache:
        _module_cache[Tsteps] = _build_module(Tsteps)
    nc = _module_cache[Tsteps]
    if Tsteps not in _exec_cache:
        _exec_cache[Tsteps] = _build_exec(nc)
    fn, in_names, out_names, zero_outs, sharding = _exec_cache[Tsteps]

    in_maps = _make_in_maps(inputs, Tsteps)
    concat_in = [
        jax.device_put(
            np.concatenate([in_maps[c][n] for c in range(NCORES)], axis=0),
            sharding)
        for n in in_names]
    for a in concat_in:
        a.block_until_ready()

    def stage_zeros():
        zs = [jax.device_put(
            np.zeros((NCORES * z.shape[0], *z.shape[1:]), z.dtype), sharding)
            for z in zero_outs]
        for z in zs:
            z.block_until_ready()
        return zs

    times = []
    out_arrs = None
    for _ in range(iters):
        zs = stage_zeros()
        t0 = time.perf_counter()
        res = fn(*concat_in, *zs)
        for r in res:
            r.block_until_ready()
        times.append(int((time.perf_counter() - t0) * 1e9))
        out_arrs = res

    res_list = []
    for c in range(NCORES):
        m = {}
        for i, name in enumerate(out_names):
            full = np.asarray(out_arrs[i])
            per = full.reshape(NCORES, -1, *full.shape[1:])
            m[name] = per[c].reshape(out_avals_shape(nc, name))
        res_list.append(m)
    out = _unpack_out(res_list, Tsteps)
    return out, times


def out_avals_shape(nc, name):
    for alloc in nc.m.functions[0].allocations:
        if (isinstance(alloc, mybir.MemoryLocationSet)
                and alloc.memorylocations[0].name == name):
            return tuple(alloc.tensor_shape)
    raise KeyError(name)
